# revision 14
# baseline (speedup 1.0000x reference)
"""DGN network (encoder MLP -> 2x TransformerConv -> per-agent readout) on TRN2.

Strategy
--------
Data-parallel over the 32 independent subgraphs: 4 graphs per NeuronCore.
The final output reads conv2 only at each graph's agent node, so per graph
only the agent's 1-hop set D1 = {agent} u N(agent) and the 2-hop set
S1 = D1 u N(D1) participate.  The host builds the index lists, gathers the
participating node features (pure indexing), and builds exact additive
adjacency masks (log-multiplicity); the device runs small dense masked
attention on the gathered sets.  All matmuls, softmaxes and aggregations
run on hardware in fp32.

This platform charges a large fixed cost per compute-engine instruction
(DMAs are comparatively free), so the kernel is shaped to minimize the
number of PE/DVE/ACT instructions:
  - conv1 scores use host-precomputed A_h = Wk_aug_h Wq_aug_h^T so all
    heads' scores come from 4 tiny MMs (t = A h2a_d1) + SC wide MMs
    (s = h2a^T t), instead of separate q/k projections + 16 per-graph MMs.
  - the 4 graphs per core are treated as one dense node set; cross-graph
    score entries are killed exactly by the additive mask (exp -> 0).
  - conv1 aggregation contracts over 128-src chunks: H x SC accumulating
    MMs produce the row-major [(g,u), (h,65)] output directly (the 65th
    column of each head block is the softmax denominator, via a ones
    column packed into the V weights).
  - single wide PSUM->SBUF evacuations (3D access patterns) instead of
    per-chunk copies; biases ride the mandatory evacuation via
    scalar.activation(bias=...).
"""

from contextlib import ExitStack

import numpy as np

BS, N, D = 32, 1000, 6
HID, H = 64, 4
OUT = 5
N_CORES = 8
GPC = BS // N_CORES  # graphs per core
NEG8 = -8.0e30       # additive mask, pre-multiplied by sqrt(HID)=8
EPS = 1.0e-30

_COMPILE_CACHE: dict = {}


# ----------------------------------------------------------------------------
# Host-side preprocessing: index sets, gathered features, masks.
# ----------------------------------------------------------------------------

def _preprocess(obs: np.ndarray, edge_index: np.ndarray):
    obs = np.asarray(obs, dtype=np.float32)
    ei = np.asarray(edge_index)
    src = ei[0].astype(np.int64)
    dst = ei[1].astype(np.int64)

    node_feats = np.ascontiguousarray(obs[:, : N * 8].reshape(BS * N, 8)[:, 2:8])
    agent = np.clip(obs[:, -1], 0, N - 1).astype(np.int32)
    agent_glob = (np.arange(BS, dtype=np.int64) * N) + agent

    # CSR of in-edges (grouped by dst), multiplicity preserved
    order = np.argsort(dst, kind="stable")
    sdst = dst[order]
    ssrc = src[order]
    bounds = np.searchsorted(sdst, np.arange(BS * N + 1))

    def in_srcs(v):
        return ssrc[bounds[v] : bounds[v + 1]]

    D1_list, S1_list = [], []
    for b in range(BS):
        a = int(agent_glob[b])
        nbr = in_srcs(a)
        others = np.unique(nbr)
        others = others[others != a]
        D1 = np.concatenate([[a], others]).astype(np.int64)
        srcs_all = np.unique(np.concatenate([in_srcs(int(u)) for u in D1]))
        extra = np.setdiff1d(srcs_all, D1)
        S1 = np.concatenate([D1, extra])
        D1_list.append(D1)
        S1_list.append(S1)

    max_d1 = max(len(x) for x in D1_list)
    max_s1 = max(len(x) for x in S1_list)
    P_D = 32 * ((max_d1 + 31) // 32)
    P_S = 32 * ((max_s1 + 31) // 32)
    assert P_D <= 32, f"agent degree too large for this layout: |D1|={max_d1}"
    assert P_S <= 512, f"2-hop set too large: |S1|={max_s1}"
    SC = GPC * P_S // 128      # 128-src chunks per core (WT = GPC*P_S)

    # Per-graph gather indices (padded with 0 -> harmless real data, masked)
    gidx = np.zeros((BS, P_S), np.int32)
    # conv1 mask, [S1 position, D1 position], pre-scaled by 8
    m1t8 = np.full((BS, P_S, P_D), NEG8, np.float32)
    # conv2 mask over D1 source positions
    m2t8 = np.full((BS, P_D), NEG8, np.float32)

    for b in range(BS):
        D1, S1 = D1_list[b], S1_list[b]
        gidx[b, : len(S1)] = S1
        pos = {int(v): i for i, v in enumerate(S1)}
        for up, u in enumerate(D1):
            s_of_u = in_srcs(int(u))
            if len(s_of_u) == 0:
                continue
            vals, cnts = np.unique(s_of_u, return_counts=True)
            for v, c in zip(vals, cnts):
                m1t8[b, pos[int(v)], up] = 8.0 * np.log(np.float32(c)) if c > 1 else 0.0
        a_srcs = in_srcs(int(D1[0]))
        if len(a_srcs):
            vals, cnts = np.unique(a_srcs, return_counts=True)
            for v, c in zip(vals, cnts):
                sp = pos[int(v)]
                assert sp < len(D1)
                m2t8[b, sp] = 8.0 * np.log(np.float32(c)) if c > 1 else 0.0

    return dict(
        node_feats=node_feats,
        gidx=gidx,
        m1t8=m1t8,
        m2t8=m2t8,
        P_D=P_D,
        P_S=P_S,
        SC=SC,
    )


def _pack_v_weights(wv, bv, n_in):
    """[n_in+1, 65*H] tile: head h -> cols [65h:65h+64] = Wv head block (with
    bias row at n_in); col 65h+64 = basis vector selecting the ones row, so
    the AV matmul also produces the softmax denominator."""
    p = np.zeros((n_in + 1, 65 * H), np.float32)
    for h in range(H):
        p[:n_in, 65 * h : 65 * h + HID] = wv[:, HID * h : HID * (h + 1)]
        p[n_in, 65 * h : 65 * h + HID] = bv[HID * h : HID * (h + 1)]
        p[n_in, 65 * h + HID] = 1.0
    return p


SMALL_SPECS = [("w1", D, HID), ("w2", HID, HID), ("b12", HID, 2)]


def _small_layout():
    layout, c = {}, 0
    for name, rows, cols in SMALL_SPECS:
        layout[name] = (c, c + cols, rows)
        c += cols
    layout["_total"] = c
    return layout


def _pack_layout(SC):
    """Column layout of the single consolidated [128, X] weight/mask pack."""
    specs = [
        ("ident", 128, 128),
        ("ones", 1, 512),
        ("onescol", 128, 1),
        ("a1t", HID + 1, H * (HID + 1)),
        ("wv1p", HID + 1, 65 * H),
        ("q2_k0", 128, H * HID), ("q2_k1", 128, H * HID), ("q2_bp", 128, 2),
        ("k2_k0", 128, H * HID), ("k2_k1", 128, H * HID), ("k2_bp", 128, 2),
        ("v2_k0", 128, H * HID), ("v2_k1", 128, H * HID), ("v2_kb", 1, H * HID),
        ("wout0", 128, OUT), ("wout1", 128, OUT), ("woutb", 1, OUT),
        ("m1big", 128, SC * 512),
        ("m2full", 128, H * GPC),
    ]
    layout, c = {}, 0
    for name, rows, cols in specs:
        layout[name] = (c, c + cols, rows)
        c += cols
    layout["_total"] = c
    return layout


def _per_core_inputs(pre, weights):
    P_D, P_S, SC = pre["P_D"], pre["P_S"], pre["SC"]
    WT = GPC * P_S
    w = weights
    layout = _pack_layout(SC)
    base = np.zeros((128, layout["_total"]), np.float32)
    slayout = _small_layout()
    small = np.zeros((128, slayout["_total"]), np.float32)

    def put(name, arr):
        c0, c1, rows = layout[name]
        assert arr.shape == (rows, c1 - c0), (name, arr.shape)
        base[:rows, c0:c1] = arr

    def puts(name, arr):
        c0, c1, rows = slayout[name]
        assert arr.shape == (rows, c1 - c0), (name, arr.shape)
        small[:rows, c0:c1] = arr

    put("ident", np.eye(128, dtype=np.float32))
    put("ones", np.ones((1, 512), np.float32))
    put("onescol", np.ones((128, 1), np.float32))

    # conv1 score kernels: lhsT for t = A_h h2a is A_h^T = Wq_aug Wk_aug^T
    wq1 = np.asarray(w["c1_wq"], np.float32)
    bq1 = np.asarray(w["c1_bq"], np.float32)
    wk1 = np.asarray(w["c1_wk"], np.float32)
    bk1 = np.asarray(w["c1_bk"], np.float32)
    a1t = np.zeros((HID + 1, H * (HID + 1)), np.float32)
    for h in range(H):
        wqa = np.vstack([wq1[:, HID * h : HID * (h + 1)],
                         bq1[None, HID * h : HID * (h + 1)]])  # [65, 64]
        wka = np.vstack([wk1[:, HID * h : HID * (h + 1)],
                         bk1[None, HID * h : HID * (h + 1)]])
        a1t[:, (HID + 1) * h : (HID + 1) * (h + 1)] = wqa @ wka.T
    put("a1t", a1t)
    put("wv1p", _pack_v_weights(
        np.asarray(w["c1_wv"], np.float32), np.asarray(w["c1_bv"], np.float32), HID
    ))

    wq2a = np.vstack([w["c2_wq"], w["c2_bq"][None, :]]).astype(np.float32)
    wk2a = np.vstack([w["c2_wk"], w["c2_bk"][None, :]]).astype(np.float32)
    wv2a = np.vstack([w["c2_wv"], w["c2_bv"][None, :]]).astype(np.float32)
    wouta = np.vstack([w["out_w"], w["out_b"][None, :]]).astype(np.float32)

    puts("w1", np.asarray(w["enc_w1"], np.float32))
    puts("w2", np.asarray(w["enc_w2"], np.float32))
    puts("b12", np.stack([w["enc_b1"], w["enc_b2"]], axis=1).astype(np.float32))
    for nm, arr in (("q2", wq2a), ("k2", wk2a)):
        put(f"{nm}_k0", arr[0:128])
        put(f"{nm}_k1", arr[128:256])
        put(f"{nm}_bp", arr[256].reshape(2, 128).T.copy())
    put("v2_k0", wv2a[0:128])
    put("v2_k1", wv2a[128:256])
    put("v2_kb", wv2a[256:257])
    put("wout0", wouta[0:128])
    put("wout1", wouta[128:256])
    put("woutb", wouta[256:257])

    in_maps = []
    for c in range(N_CORES):
        pack = base.copy()

        def putc(name, arr):
            c0, c1, rows = layout[name]
            assert arr.shape == (rows, c1 - c0), (name, arr.shape)
            pack[:rows, c0:c1] = arr

        gs = slice(c * GPC, (c + 1) * GPC)
        # host-side gather: featsT [7, WT], row 6 = ones (bias row)
        flat = pre["gidx"][gs].reshape(-1).astype(np.int64)   # [WT]
        gf = pre["node_feats"][flat]                          # [WT, 6]
        featsT = np.ones((D + 1, WT), np.float32)
        featsT[0:D] = gf.T
        # conv1 mask, chunk-dense: [128 src-in-chunk, (chunk, head, g, u)]
        m1big = np.full((128, SC * 512), NEG8, np.float32)
        for ck in range(SC):
            for p in range(128):
                s = ck * 128 + p
                if s >= WT:
                    break
                g_s, pos = divmod(s, P_S)
                row = pre["m1t8"][c * GPC + g_s][pos]         # [P_D]
                for h in range(H):
                    col0 = ck * 512 + h * 128 + g_s * P_D
                    m1big[p, col0 : col0 + P_D] = row
        putc("m1big", m1big)
        # conv2 mask [P_D (D1 source position), (h, g)]
        m2full = np.full((128, H * GPC), NEG8, np.float32)
        for g in range(GPC):
            for h in range(H):
                m2full[:P_D, h * GPC + g] = pre["m2t8"][c * GPC + g]
        putc("m2full", m2full)
        in_maps.append({"featsT": featsT, "wpack": pack, "wsmall": small})
    return in_maps


# ----------------------------------------------------------------------------
# Device program
# ----------------------------------------------------------------------------

def _build_program(P_D, P_S, SC, reps=1, enable_asserts=False):
    import concourse.bass as bass
    import concourse.tile as tile
    from concourse import bacc, mybir

    f32 = mybir.dt.float32
    AF = mybir.ActivationFunctionType

    assert P_D == 32 and GPC == 4
    UW = GPC * P_D              # packed conv1-dst width = 128
    WT = GPC * P_S              # gathered-node columns
    assert WT % 128 == 0 and SC == WT // 128
    assert WT <= 512, f"encoder single-chunk layout needs WT<=512, got {WT}"

    nc = bacc.Bacc(
        "TRN2",
        target_bir_lowering=False,
        debug=False,
        enable_asserts=enable_asserts,
        num_devices=N_CORES,
    )

    layout = _pack_layout(SC)
    tot_cols = layout["_total"]
    featsT = nc.dram_tensor("featsT", (D + 1, WT), f32, kind="ExternalInput").ap()
    wpack = nc.dram_tensor("wpack", (128, tot_cols), f32, kind="ExternalInput").ap()
    slayout = _small_layout()
    wsmall = nc.dram_tensor(
        "wsmall", (128, slayout["_total"]), f32, kind="ExternalInput"
    ).ap()
    out = nc.dram_tensor("out", (GPC, OUT), f32, kind="ExternalOutput").ap()

    with tile.TileContext(nc) as tc, ExitStack() as ctx:
        cp = ctx.enter_context(tc.tile_pool(name="const", bufs=1))
        wp = ctx.enter_context(tc.tile_pool(name="work", bufs=2))
        pp = ctx.enter_context(tc.tile_pool(name="psum", bufs=1, space="PSUM"))

        def ctile(shape, name, dt=f32):
            return cp.tile(shape, dt, tag=name, name=name)

        wsm = ctile([128, slayout["_total"]], "wsm")
        nc.sync.dma_start(wsm[:], wsmall)
        wpk = ctile([128, tot_cols], "wpk")
        nc.sync.dma_start(wpk[:], wpack)

        def wsl(name, rows):
            c0, c1, _r = layout[name]
            return wpk[0:rows, c0:c1]

        def ssl(name, rows):
            c0, c1, _r = slayout[name]
            return wsm[0:rows, c0:c1]

        ident = wsl("ident", 128)
        ones_row = wsl("ones", 1)
        ones_col = wsl("onescol", 128)
        w1_sb = ssl("w1", D)
        w2_sb = ssl("w2", HID)
        b12_sb = ssl("b12", HID)
        a1t_sb = wsl("a1t", HID + 1)
        wv1_sb = wsl("wv1p", HID + 1)
        w2ch = {
            nm: (wsl(f"{nm}_k0", 128), wsl(f"{nm}_k1", 128), wsl(f"{nm}_bp", 128))
            for nm in ("q2", "k2")
        }
        w2ch["v2"] = (wsl("v2_k0", 128), wsl("v2_k1", 128), wsl("v2_kb", 1))
        wout0 = wsl("wout0", 128)
        wout1 = wsl("wout1", 128)
        woutb = wsl("woutb", 1)
        m1_sb = wsl("m1big", 128)
        m2_sb = wsl("m2full", 128)
        ones_c0, _oc1, _ocr = layout["ones"]

        for r in range(reps):
            def wtile(shape, name, dt=f32):
                return wp.tile(shape, dt, tag=name, name=name)

            def ptile(shape, name, tag, bufs=2):
                return pp.tile(shape, f32, tag=tag, name=name, bufs=bufs)

            # ---- load gathered features (host did the gather) ----
            fT = wtile([D + 1, WT], "fT")
            nc.sync.dma_start(fT[:], featsT)

            # ---- encoder: 2 MMs + 2 ACTs over all WT columns at once ----
            h2a = wtile([HID + 1, WT], "h2a")
            # ones row for the bias/denominator tricks comes via DMA
            nc.sync.dma_start(
                h2a[HID : HID + 1, :], wpack[0:1, ones_c0 : ones_c0 + WT]
            )
            p1 = ptile([HID, 512], "h1ps", tag="one")
            nc.tensor.matmul(p1[:, 0:WT], w1_sb[:], fT[0:D, :])
            h1T = wtile([HID, WT], "h1T")
            nc.scalar.activation(h1T[:], p1[:, 0:WT], AF.Relu, bias=b12_sb[:, 0:1])
            p2 = ptile([HID, 512], "h2ps", tag="one")
            nc.tensor.matmul(p2[:, 0:WT], w2_sb[:], h1T[:])
            nc.scalar.activation(
                h2a[0:HID, :], p2[:, 0:WT], AF.Relu, bias=b12_sb[:, 1:2]
            )
            h2a_d1 = h2a.rearrange("p (g c) -> p g c", g=GPC)[:, :, 0:P_D]

            # ---- conv1 scores: t = A_h h2a_d1 (4 MMs), s = h2a^T t ----
            tps = ptile([HID + 1, 512], "tps", tag="one")
            for h in range(H):
                nc.tensor.matmul(
                    tps[:, UW * h : UW * (h + 1)],
                    a1t_sb[:, (HID + 1) * h : (HID + 1) * (h + 1)],
                    h2a_d1,
                )
            t_sb = wtile([HID + 1, 512], "t_sb")
            nc.scalar.copy(t_sb[:], tps[:])

            sps = ptile([128, SC * 512], "sps", tag="big")
            for c in range(SC):
                nc.tensor.matmul(
                    sps[:, 512 * c : 512 * (c + 1)],
                    h2a[:, 128 * c : 128 * (c + 1)],
                    t_sb[:],
                )
            e_sb = wtile([128, SC * 512], "e_sb")
            nc.vector.tensor_add(e_sb[:], sps[:], m1_sb[:, 0 : SC * 512])
            nc.scalar.activation(e_sb[:], e_sb[:], AF.Exp, scale=0.125)

            # ---- conv1 v (per src chunk) ----
            vps = ptile([128, SC * 512], "vps", tag="big")
            for c in range(SC):
                nc.tensor.matmul(
                    vps[:, 512 * c : 512 * c + 65 * H],
                    h2a[:, 128 * c : 128 * (c + 1)],
                    wv1_sb[:],
                )
            v_sb = wtile([128, SC * 65 * H], "v_sb")
            nc.scalar.copy(
                v_sb.rearrange("p (c w) -> p c w", c=SC),
                vps.rearrange("p (c w) -> p c w", c=SC)[:, :, 0 : 65 * H],
            )

            # ---- conv1 aggregation: H x SC accumulating MMs ----
            o1 = ptile([128, 512], "o1", tag="one")
            for h in range(H):
                for c in range(SC):
                    nc.tensor.matmul(
                        o1[:, 65 * h : 65 * h + 65],
                        e_sb[:, 512 * c + UW * h : 512 * c + UW * (h + 1)],
                        v_sb[:, 65 * H * c + 65 * h : 65 * H * c + 65 * h + 65],
                        start=(c == 0),
                        stop=(c == SC - 1),
                    )

            # normalization: per-partition (per dst node) activation scale
            z1 = wtile([128, H], "z1")
            nc.vector.tensor_scalar_add(z1[:], o1[:, HID : 65 * H : 65], EPS)
            rz1 = wtile([128, H], "rz1")
            nc.vector.reciprocal(rz1[:], z1[:])
            h1cRM = wtile([128, H * HID], "h1cRM")
            for h in range(H):
                nc.scalar.activation(
                    h1cRM[:, HID * h : HID * (h + 1)],
                    o1[:, 65 * h : 65 * h + HID],
                    AF.Relu,
                    scale=rz1[:, h : h + 1],
                )

            # transpose h1c to feature-major for the conv2 projections
            h1cT = []
            for mc in range(2):
                tp = ptile([128, 128], "h1cTps", tag="one")
                nc.tensor.transpose(
                    tp[:], h1cRM[:, 128 * mc : 128 * (mc + 1)], ident[:]
                )
                t = wtile([128, 128], f"h1cT_{mc}")
                nc.scalar.copy(t[:], tp[:])
                h1cT.append(t)

            # ---- conv2 projections (agents / D1 nodes only) ----
            agent_cols_a = h1cT[0][:, 0:UW:P_D]
            agent_cols_b = h1cT[1][:, 0:UW:P_D]

            def proj2(nm, rhs_a, rhs_b, width, name):
                k0, k1_, bp = w2ch[nm]
                outt = []
                for mc in range(2):
                    ps = ptile([128, width], f"{name}ps_{mc}", tag="one")
                    nc.tensor.matmul(
                        ps[:], k0[:, mc * 128 : (mc + 1) * 128],
                        rhs_a, start=True, stop=False,
                    )
                    nc.tensor.matmul(
                        ps[:], k1_[:, mc * 128 : (mc + 1) * 128],
                        rhs_b, start=False, stop=True,
                    )
                    t = wtile([128, width], f"{name}_{mc}")
                    nc.scalar.activation(
                        t[:], ps[:], AF.Identity, bias=bp[:, mc : mc + 1]
                    )
                    outt.append(t)
                return outt

            q2T = proj2("q2", agent_cols_a, agent_cols_b, GPC, "q2T")
            k2T = proj2("k2", h1cT[0][:], h1cT[1][:], UW, "k2T")

            # v2 for all D1 nodes at once: [128, 256]
            vps2 = ptile([128, H * HID], "v2ps", tag="one")
            vk0, vk1, vkb = w2ch["v2"]
            nc.tensor.matmul(vps2[:], h1cT[0][:], vk0[:], start=True, stop=False)
            nc.tensor.matmul(vps2[:], h1cT[1][:], vk1[:], start=False, stop=False)
            nc.tensor.matmul(
                vps2[:], ones_row[:, 0:UW], vkb[:], start=False, stop=True
            )
            v2g = []
            for g in range(GPC):
                t = wtile([P_D, H * HID], f"v2_{g}")
                nc.vector.tensor_copy(t[:], vps2[g * P_D : (g + 1) * P_D, :])
                v2g.append(t)

            # ---- conv2 attention: merged scores [128 (all D1), H*GPC] ----
            s2ps = ptile([128, H * GPC], "s2ps", tag="one")
            for h in range(H):
                mc, hr = divmod(h, 2)
                nc.tensor.matmul(
                    s2ps[:, h * GPC : (h + 1) * GPC],
                    k2T[mc][hr * HID : (hr + 1) * HID, :],
                    q2T[mc][hr * HID : (hr + 1) * HID, :],
                )
            e2 = wtile([P_D, H * GPC], "e2")
            for g in range(GPC):
                nc.vector.tensor_add(
                    e2[:, g : H * GPC : GPC],
                    s2ps[g * P_D : (g + 1) * P_D, g : H * GPC : GPC],
                    m2_sb[0:P_D, g : H * GPC : GPC],
                )
            nc.scalar.activation(e2[:], e2[:], AF.Exp, scale=0.125)

            # aggregate: per (graph, head-pair) one [32, 128] x [32, 2]
            # matmul. Column 4g+2mc+hh holds head (2mc+hh)'s output in rows
            # [64hh, 64hh+64); the other half is cross-head garbage, unread.
            o2 = ptile([128, H * GPC], "o2", tag="one")
            for g in range(GPC):
                for mc in range(2):
                    c0 = 4 * g + 2 * mc
                    nc.tensor.matmul(
                        o2[:, c0 : c0 + 2],
                        v2g[g][:, mc * 128 : (mc + 1) * 128],
                        e2[:, 8 * mc + g : 8 * mc + g + 5 : 4],
                    )
            z2ps = ptile([1, H * GPC], "z2ps", tag="one")
            nc.tensor.matmul(z2ps[:], ones_col[0:P_D, :], e2[:])
            z2row = wtile([1, H * GPC], "z2row")
            nc.vector.tensor_scalar_add(z2row[:], z2ps[:], EPS)
            rz2row = wtile([1, H * GPC], "rz2row")
            nc.vector.reciprocal(rz2row[:], z2row[:])
            rz2ps = ptile([HID, H * GPC], "rz2ps", tag="one")
            nc.tensor.matmul(rz2ps[:], ones_row[:, 0:HID], rz2row[:])
            rl_all = wtile([128, H * GPC], "rl_all")
            nc.scalar.activation(rl_all[:], o2[:], AF.Relu)
            h2T_f = []
            for mc in range(2):
                t = wtile([128, GPC], f"h2T_{mc}")
                for hh in range(2):
                    h = mc * 2 + hh
                    nc.vector.tensor_mul(
                        t[hh * HID : (hh + 1) * HID, :],
                        rl_all[
                            hh * HID : (hh + 1) * HID,
                            2 * mc + hh : H * GPC : 4,
                        ],
                        rz2ps[0:HID, h * GPC : (h + 1) * GPC],
                    )
                h2T_f.append(t)

            # ---- readout: out = h2 @ out_w + out_b ----
            ops = ptile([GPC, OUT], "outps", tag="one")
            nc.tensor.matmul(ops[:], h2T_f[0][:], wout0[:], start=True, stop=False)
            nc.tensor.matmul(ops[:], h2T_f[1][:], wout1[:], start=False, stop=False)
            nc.tensor.matmul(
                ops[:], ones_row[:, 0:GPC], woutb[:], start=False, stop=True
            )
            osb = wtile([GPC, OUT], "osb")
            nc.vector.tensor_copy(osb[:], ops[:])
            nc.sync.dma_start(out, osb[:])

    nc.compile()
    return nc


# ----------------------------------------------------------------------------
# Entry point
# ----------------------------------------------------------------------------

def kernel(**inputs) -> np.ndarray:
    from concourse import bass_utils

    obs = np.asarray(inputs["obs"], np.float32)
    pre = _preprocess(obs, inputs["edge_index"])
    in_maps = _per_core_inputs(pre, inputs)

    key = (pre["P_D"], pre["P_S"])
    if key not in _COMPILE_CACHE:
        _COMPILE_CACHE[key] = _build_program(pre["P_D"], pre["P_S"], pre["SC"])
    nc = _COMPILE_CACHE[key]

    res = bass_utils.run_bass_kernel_spmd(nc, in_maps, core_ids=list(range(N_CORES)))
    out = np.concatenate([res.results[c]["out"] for c in range(N_CORES)], axis=0)
    return out.astype(np.float32)


# revision 17
# speedup vs baseline: 26.7408x; 26.7408x over previous
"""DGN network (encoder MLP -> 2x TransformerConv -> per-agent readout) on TRN2.

Strategy
--------
Data-parallel over the 32 independent subgraphs: 4 graphs per NeuronCore.
The final output reads conv2 only at each graph's agent node, so per graph
only the agent's 1-hop set D1 = {agent} u N(agent) and the 2-hop set
S1 = D1 u N(D1) participate.  The host builds the index lists, gathers the
participating node features (pure indexing), and builds exact additive
adjacency masks (log-multiplicity); the device runs small dense masked
attention on the gathered sets.  All matmuls, softmaxes and aggregations
run on hardware in fp32.

This platform charges a large fixed cost per compute-engine instruction
(DMAs are comparatively free), so the kernel is shaped to minimize the
number of PE/DVE/ACT instructions:
  - conv1 scores use host-precomputed A_h = Wk_aug_h Wq_aug_h^T so all
    heads' scores come from 4 tiny MMs (t = A h2a_d1) + SC wide MMs
    (s = h2a^T t), instead of separate q/k projections + 16 per-graph MMs.
  - the 4 graphs per core are treated as one dense node set; cross-graph
    score entries are killed exactly by the additive mask (exp -> 0).
  - conv1 aggregation contracts over 128-src chunks: H x SC accumulating
    MMs produce the row-major [(g,u), (h,65)] output directly (the 65th
    column of each head block is the softmax denominator, via a ones
    column packed into the V weights).
  - single wide PSUM->SBUF evacuations (3D access patterns) instead of
    per-chunk copies; biases ride the mandatory evacuation via
    scalar.activation(bias=...).
"""

from contextlib import ExitStack

import numpy as np

BS, N, D = 32, 1000, 6
HID, H = 64, 4
OUT = 5
N_CORES = 8
GPC = BS // N_CORES  # graphs per core
NEG8 = -8.0e30       # additive mask, pre-multiplied by sqrt(HID)=8
EPS = 1.0e-30

_COMPILE_CACHE: dict = {}


# ----------------------------------------------------------------------------
# Host-side preprocessing: index sets, gathered features, masks.
# ----------------------------------------------------------------------------

def _preprocess(obs: np.ndarray, edge_index: np.ndarray):
    obs = np.asarray(obs, dtype=np.float32)
    ei = np.asarray(edge_index)
    src = ei[0].astype(np.int64)
    dst = ei[1].astype(np.int64)

    node_feats = np.ascontiguousarray(obs[:, : N * 8].reshape(BS * N, 8)[:, 2:8])
    agent = np.clip(obs[:, -1], 0, N - 1).astype(np.int32)
    agent_glob = (np.arange(BS, dtype=np.int64) * N) + agent

    # CSR of in-edges (grouped by dst), multiplicity preserved
    order = np.argsort(dst, kind="stable")
    sdst = dst[order]
    ssrc = src[order]
    bounds = np.searchsorted(sdst, np.arange(BS * N + 1))

    def in_srcs(v):
        return ssrc[bounds[v] : bounds[v + 1]]

    D1_list, S1_list = [], []
    for b in range(BS):
        a = int(agent_glob[b])
        nbr = in_srcs(a)
        others = np.unique(nbr)
        others = others[others != a]
        D1 = np.concatenate([[a], others]).astype(np.int64)
        srcs_all = np.unique(np.concatenate([in_srcs(int(u)) for u in D1]))
        extra = np.setdiff1d(srcs_all, D1)
        S1 = np.concatenate([D1, extra])
        D1_list.append(D1)
        S1_list.append(S1)

    max_d1 = max(len(x) for x in D1_list)
    max_s1 = max(len(x) for x in S1_list)
    P_D = 32 * ((max_d1 + 31) // 32)
    P_S = 32 * ((max_s1 + 31) // 32)
    assert P_D <= 32, f"agent degree too large for this layout: |D1|={max_d1}"
    assert P_S <= 512, f"2-hop set too large: |S1|={max_s1}"
    SC = GPC * P_S // 128      # 128-src chunks per core (WT = GPC*P_S)

    # Per-graph gather indices (padded with 0 -> harmless real data, masked)
    gidx = np.zeros((BS, P_S), np.int32)
    # conv1 mask, [S1 position, D1 position], pre-scaled by 8
    m1t8 = np.full((BS, P_S, P_D), NEG8, np.float32)
    # conv2 mask over D1 source positions
    m2t8 = np.full((BS, P_D), NEG8, np.float32)

    for b in range(BS):
        D1, S1 = D1_list[b], S1_list[b]
        gidx[b, : len(S1)] = S1
        pos = {int(v): i for i, v in enumerate(S1)}
        for up, u in enumerate(D1):
            s_of_u = in_srcs(int(u))
            if len(s_of_u) == 0:
                continue
            vals, cnts = np.unique(s_of_u, return_counts=True)
            for v, c in zip(vals, cnts):
                m1t8[b, pos[int(v)], up] = 8.0 * np.log(np.float32(c)) if c > 1 else 0.0
        a_srcs = in_srcs(int(D1[0]))
        if len(a_srcs):
            vals, cnts = np.unique(a_srcs, return_counts=True)
            for v, c in zip(vals, cnts):
                sp = pos[int(v)]
                assert sp < len(D1)
                m2t8[b, sp] = 8.0 * np.log(np.float32(c)) if c > 1 else 0.0

    return dict(
        node_feats=node_feats,
        gidx=gidx,
        m1t8=m1t8,
        m2t8=m2t8,
        P_D=P_D,
        P_S=P_S,
        SC=SC,
    )


def _pack_v_weights(wv, bv, n_in):
    """[n_in+1, 65*H] tile: head h -> cols [65h:65h+64] = Wv head block (with
    bias row at n_in); col 65h+64 = basis vector selecting the ones row, so
    the AV matmul also produces the softmax denominator."""
    p = np.zeros((n_in + 1, 65 * H), np.float32)
    for h in range(H):
        p[:n_in, 65 * h : 65 * h + HID] = wv[:, HID * h : HID * (h + 1)]
        p[n_in, 65 * h : 65 * h + HID] = bv[HID * h : HID * (h + 1)]
        p[n_in, 65 * h + HID] = 1.0
    return p


SMALL_SPECS = [("w1", D, HID), ("w2", HID, HID), ("b12", HID, 2)]


def _small_layout():
    layout, c = {}, 0
    for name, rows, cols in SMALL_SPECS:
        layout[name] = (c, c + cols, rows)
        c += cols
    layout["_total"] = c
    return layout


def _pack_layout(SC):
    """Column layout of the single consolidated [128, X] weight/mask pack."""
    specs = [
        ("ident", 128, 128),
        ("ones", 1, 512),
        ("onescol", 128, 1),
        ("a1t", HID + 1, H * (HID + 1)),
        ("wv1p", HID + 1, 65 * H),
        ("q2_k0", 128, H * HID), ("q2_k1", 128, H * HID), ("q2_bp", 128, 2),
        ("k2_k0", 128, H * HID), ("k2_k1", 128, H * HID), ("k2_bp", 128, 2),
        ("v2_k0", 128, H * HID), ("v2_k1", 128, H * HID), ("v2_kb", 1, H * HID),
        ("wout0", 128, OUT), ("wout1", 128, OUT), ("woutb", 1, OUT),
        ("m1big", 128, SC * 512),
        ("m2full", 128, H * GPC),
    ]
    layout, c = {}, 0
    for name, rows, cols in specs:
        layout[name] = (c, c + cols, rows)
        c += cols
    layout["_total"] = c
    return layout


def _per_core_inputs(pre, weights):
    P_D, P_S, SC = pre["P_D"], pre["P_S"], pre["SC"]
    WT = GPC * P_S
    w = weights
    layout = _pack_layout(SC)
    base = np.zeros((128, layout["_total"]), np.float32)
    slayout = _small_layout()
    small = np.zeros((128, slayout["_total"]), np.float32)

    def put(name, arr):
        c0, c1, rows = layout[name]
        assert arr.shape == (rows, c1 - c0), (name, arr.shape)
        base[:rows, c0:c1] = arr

    def puts(name, arr):
        c0, c1, rows = slayout[name]
        assert arr.shape == (rows, c1 - c0), (name, arr.shape)
        small[:rows, c0:c1] = arr

    put("ident", np.eye(128, dtype=np.float32))
    put("ones", np.ones((1, 512), np.float32))
    put("onescol", np.ones((128, 1), np.float32))

    # conv1 score kernels: lhsT for t = A_h h2a is A_h^T = Wq_aug Wk_aug^T
    wq1 = np.asarray(w["c1_wq"], np.float32)
    bq1 = np.asarray(w["c1_bq"], np.float32)
    wk1 = np.asarray(w["c1_wk"], np.float32)
    bk1 = np.asarray(w["c1_bk"], np.float32)
    a1t = np.zeros((HID + 1, H * (HID + 1)), np.float32)
    for h in range(H):
        wqa = np.vstack([wq1[:, HID * h : HID * (h + 1)],
                         bq1[None, HID * h : HID * (h + 1)]])  # [65, 64]
        wka = np.vstack([wk1[:, HID * h : HID * (h + 1)],
                         bk1[None, HID * h : HID * (h + 1)]])
        a1t[:, (HID + 1) * h : (HID + 1) * (h + 1)] = wqa @ wka.T
    put("a1t", a1t)
    put("wv1p", _pack_v_weights(
        np.asarray(w["c1_wv"], np.float32), np.asarray(w["c1_bv"], np.float32), HID
    ))

    wq2a = np.vstack([w["c2_wq"], w["c2_bq"][None, :]]).astype(np.float32)
    wk2a = np.vstack([w["c2_wk"], w["c2_bk"][None, :]]).astype(np.float32)
    wv2a = np.vstack([w["c2_wv"], w["c2_bv"][None, :]]).astype(np.float32)
    wouta = np.vstack([w["out_w"], w["out_b"][None, :]]).astype(np.float32)

    puts("w1", np.asarray(w["enc_w1"], np.float32))
    puts("w2", np.asarray(w["enc_w2"], np.float32))
    puts("b12", np.stack([w["enc_b1"], w["enc_b2"]], axis=1).astype(np.float32))
    for nm, arr in (("q2", wq2a), ("k2", wk2a)):
        put(f"{nm}_k0", arr[0:128])
        put(f"{nm}_k1", arr[128:256])
        put(f"{nm}_bp", arr[256].reshape(2, 128).T.copy())
    put("v2_k0", wv2a[0:128])
    put("v2_k1", wv2a[128:256])
    put("v2_kb", wv2a[256:257])
    put("wout0", wouta[0:128])
    put("wout1", wouta[128:256])
    put("woutb", wouta[256:257])

    in_maps = []
    for c in range(N_CORES):
        pack = base.copy()

        def putc(name, arr):
            c0, c1, rows = layout[name]
            assert arr.shape == (rows, c1 - c0), (name, arr.shape)
            pack[:rows, c0:c1] = arr

        gs = slice(c * GPC, (c + 1) * GPC)
        # host-side gather: featsT [7, WT], row 6 = ones (bias row)
        flat = pre["gidx"][gs].reshape(-1).astype(np.int64)   # [WT]
        gf = pre["node_feats"][flat]                          # [WT, 6]
        featsT = np.ones((D + 1, WT), np.float32)
        featsT[0:D] = gf.T
        # conv1 mask, chunk-dense: [128 src-in-chunk, (chunk, head, g, u)]
        m1big = np.full((128, SC * 512), NEG8, np.float32)
        for ck in range(SC):
            for p in range(128):
                s = ck * 128 + p
                if s >= WT:
                    break
                g_s, pos = divmod(s, P_S)
                row = pre["m1t8"][c * GPC + g_s][pos]         # [P_D]
                for h in range(H):
                    col0 = ck * 512 + h * 128 + g_s * P_D
                    m1big[p, col0 : col0 + P_D] = row
        putc("m1big", m1big)
        # conv2 mask [128 (g', u'), (h, g)]: valid only where g' == g
        m2full = np.full((128, H * GPC), NEG8, np.float32)
        for g in range(GPC):
            for h in range(H):
                m2full[g * P_D : (g + 1) * P_D, h * GPC + g] = (
                    pre["m2t8"][c * GPC + g]
                )
        putc("m2full", m2full)
        in_maps.append({"featsT": featsT, "wpack": pack, "wsmall": small})
    return in_maps


# ----------------------------------------------------------------------------
# Device program
# ----------------------------------------------------------------------------

def _build_program(P_D, P_S, SC, reps=1, enable_asserts=False, hwloop=True):
    import concourse.bass as bass
    import concourse.tile as tile
    from concourse import bacc, mybir

    f32 = mybir.dt.float32
    AF = mybir.ActivationFunctionType

    assert P_D == 32 and GPC == 4
    UW = GPC * P_D              # packed conv1-dst width = 128
    WT = GPC * P_S              # gathered-node columns
    assert WT % 128 == 0 and SC == WT // 128
    assert WT <= 512, f"encoder single-chunk layout needs WT<=512, got {WT}"

    nc = bacc.Bacc(
        "TRN2",
        target_bir_lowering=False,
        debug=False,
        enable_asserts=enable_asserts,
        num_devices=N_CORES,
    )

    layout = _pack_layout(SC)
    tot_cols = layout["_total"]
    featsT = nc.dram_tensor("featsT", (D + 1, WT), f32, kind="ExternalInput").ap()
    wpack = nc.dram_tensor("wpack", (128, tot_cols), f32, kind="ExternalInput").ap()
    slayout = _small_layout()
    wsmall = nc.dram_tensor(
        "wsmall", (128, slayout["_total"]), f32, kind="ExternalInput"
    ).ap()
    out = nc.dram_tensor("out", (GPC, OUT), f32, kind="ExternalOutput").ap()

    with tile.TileContext(nc) as tc, ExitStack() as ctx:
        cp = ctx.enter_context(tc.tile_pool(name="const", bufs=1))
        wp = ctx.enter_context(tc.tile_pool(name="work", bufs=2))
        pp = ctx.enter_context(tc.tile_pool(name="psum", bufs=1, space="PSUM"))

        def ctile(shape, name, dt=f32):
            return cp.tile(shape, dt, tag=name, name=name)

        wsm = ctile([128, slayout["_total"]], "wsm")
        nc.sync.dma_start(wsm[:], wsmall)
        wpk = ctile([128, tot_cols], "wpk")
        nc.sync.dma_start(wpk[:], wpack)

        def wsl(name, rows):
            c0, c1, _r = layout[name]
            return wpk[0:rows, c0:c1]

        def ssl(name, rows):
            c0, c1, _r = slayout[name]
            return wsm[0:rows, c0:c1]

        ident = wsl("ident", 128)
        ones_row = wsl("ones", 1)
        ones_col = wsl("onescol", 128)
        w1_sb = ssl("w1", D)
        w2_sb = ssl("w2", HID)
        b12_sb = ssl("b12", HID)
        a1t_sb = wsl("a1t", HID + 1)
        wv1_sb = wsl("wv1p", HID + 1)
        w2ch = {
            nm: (wsl(f"{nm}_k0", 128), wsl(f"{nm}_k1", 128), wsl(f"{nm}_bp", 128))
            for nm in ("q2", "k2")
        }
        w2ch["v2"] = (wsl("v2_k0", 128), wsl("v2_k1", 128), wsl("v2_kb", 1))
        wout0 = wsl("wout0", 128)
        wout1 = wsl("wout1", 128)
        woutb = wsl("woutb", 1)
        m1_sb = wsl("m1big", 128)
        m2_sb = wsl("m2full", 128)
        ones_c0, _oc1, _ocr = layout["ones"]

        def _rep_body():
            def wtile(shape, name, dt=f32):
                return wp.tile(shape, dt, tag=name, name=name)

            def ptile(shape, name, tag, bufs=2):
                return pp.tile(shape, f32, tag=tag, name=name, bufs=bufs)

            # ---- load gathered features (host did the gather) ----
            fT = wtile([D + 1, WT], "fT")
            nc.sync.dma_start(fT[:], featsT)

            # ---- encoder: 2 MMs + 2 ACTs over all WT columns at once ----
            h2a = wtile([HID + 1, WT], "h2a")
            # ones row for the bias/denominator tricks comes via DMA
            nc.sync.dma_start(
                h2a[HID : HID + 1, :], wpack[0:1, ones_c0 : ones_c0 + WT]
            )
            p1 = ptile([HID, 512], "h1ps", tag="one")
            nc.tensor.matmul(p1[:, 0:WT], w1_sb[:], fT[0:D, :])
            h1T = wtile([HID, WT], "h1T")
            nc.scalar.activation(h1T[:], p1[:, 0:WT], AF.Relu, bias=b12_sb[:, 0:1])
            p2 = ptile([HID, 512], "h2ps", tag="one")
            nc.tensor.matmul(p2[:, 0:WT], w2_sb[:], h1T[:])
            nc.scalar.activation(
                h2a[0:HID, :], p2[:, 0:WT], AF.Relu, bias=b12_sb[:, 1:2]
            )
            h2a_d1 = h2a.rearrange("p (g c) -> p g c", g=GPC)[:, :, 0:P_D]

            # ---- conv1 scores: t = A_h h2a_d1 (4 MMs), s = h2a^T t ----
            tps = ptile([HID + 1, 512], "tps", tag="one")
            for h in range(H):
                nc.tensor.matmul(
                    tps[:, UW * h : UW * (h + 1)],
                    a1t_sb[:, (HID + 1) * h : (HID + 1) * (h + 1)],
                    h2a_d1,
                )
            t_sb = wtile([HID + 1, 512], "t_sb")
            nc.scalar.copy(t_sb[:], tps[:])

            sps = ptile([128, SC * 512], "sps", tag="big")
            for c in range(SC):
                nc.tensor.matmul(
                    sps[:, 512 * c : 512 * (c + 1)],
                    h2a[:, 128 * c : 128 * (c + 1)],
                    t_sb[:],
                )
            e_sb = wtile([128, SC * 512], "e_sb")
            nc.vector.tensor_add(e_sb[:], sps[:], m1_sb[:, 0 : SC * 512])
            nc.scalar.activation(e_sb[:], e_sb[:], AF.Exp, scale=0.125)

            # ---- conv1 v (per src chunk) ----
            vps = ptile([128, SC * 512], "vps", tag="big")
            for c in range(SC):
                nc.tensor.matmul(
                    vps[:, 512 * c : 512 * c + 65 * H],
                    h2a[:, 128 * c : 128 * (c + 1)],
                    wv1_sb[:],
                )
            v_sb = wtile([128, SC * 65 * H], "v_sb")
            nc.scalar.copy(
                v_sb.rearrange("p (c w) -> p c w", c=SC),
                vps.rearrange("p (c w) -> p c w", c=SC)[:, :, 0 : 65 * H],
            )

            # ---- conv1 aggregation: H x SC accumulating MMs ----
            o1 = ptile([128, 512], "o1", tag="one")
            for h in range(H):
                for c in range(SC):
                    nc.tensor.matmul(
                        o1[:, 65 * h : 65 * h + 65],
                        e_sb[:, 512 * c + UW * h : 512 * c + UW * (h + 1)],
                        v_sb[:, 65 * H * c + 65 * h : 65 * H * c + 65 * h + 65],
                        start=(c == 0),
                        stop=(c == SC - 1),
                    )

            # normalization: per-partition (per dst node) activation scale
            z1 = wtile([128, H], "z1")
            nc.vector.tensor_scalar_add(z1[:], o1[:, HID : 65 * H : 65], EPS)
            rz1 = wtile([128, H], "rz1")
            nc.vector.reciprocal(rz1[:], z1[:])
            h1cRM = wtile([128, H * HID], "h1cRM")
            for h in range(H):
                nc.scalar.activation(
                    h1cRM[:, HID * h : HID * (h + 1)],
                    o1[:, 65 * h : 65 * h + HID],
                    AF.Relu,
                    scale=rz1[:, h : h + 1],
                )

            # transpose h1c to feature-major for the conv2 projections
            h1cT = []
            for mc in range(2):
                tp = ptile([128, 128], "h1cTps", tag="one")
                nc.tensor.transpose(
                    tp[:], h1cRM[:, 128 * mc : 128 * (mc + 1)], ident[:]
                )
                t = wtile([128, 128], f"h1cT_{mc}")
                nc.scalar.copy(t[:], tp[:])
                h1cT.append(t)

            # ---- conv2 projections (agents / D1 nodes only) ----
            agent_cols_a = h1cT[0][:, 0:UW:P_D]
            agent_cols_b = h1cT[1][:, 0:UW:P_D]

            def proj2(nm, rhs_a, rhs_b, width, name):
                k0, k1_, bp = w2ch[nm]
                outt = []
                for mc in range(2):
                    ps = ptile([128, width], f"{name}ps_{mc}", tag="one")
                    nc.tensor.matmul(
                        ps[:], k0[:, mc * 128 : (mc + 1) * 128],
                        rhs_a, start=True, stop=False,
                    )
                    nc.tensor.matmul(
                        ps[:], k1_[:, mc * 128 : (mc + 1) * 128],
                        rhs_b, start=False, stop=True,
                    )
                    t = wtile([128, width], f"{name}_{mc}")
                    nc.scalar.activation(
                        t[:], ps[:], AF.Identity, bias=bp[:, mc : mc + 1]
                    )
                    outt.append(t)
                return outt

            q2T = proj2("q2", agent_cols_a, agent_cols_b, GPC, "q2T")
            k2T = proj2("k2", h1cT[0][:], h1cT[1][:], UW, "k2T")

            # v2 for all D1 nodes at once: [128, 256]
            vps2 = ptile([128, H * HID], "v2ps", tag="one")
            vk0, vk1, vkb = w2ch["v2"]
            nc.tensor.matmul(vps2[:], h1cT[0][:], vk0[:], start=True, stop=False)
            nc.tensor.matmul(vps2[:], h1cT[1][:], vk1[:], start=False, stop=False)
            nc.tensor.matmul(
                vps2[:], ones_row[:, 0:UW], vkb[:], start=False, stop=True
            )
            v2g = []
            for g in range(GPC):
                t = wtile([P_D, H * HID], f"v2_{g}")
                nc.vector.tensor_copy(t[:], vps2[g * P_D : (g + 1) * P_D, :])
                v2g.append(t)

            # ---- conv2 attention: merged scores [128 (all D1), H*GPC] ----
            s2ps = ptile([128, H * GPC], "s2ps", tag="one")
            for h in range(H):
                mc, hr = divmod(h, 2)
                nc.tensor.matmul(
                    s2ps[:, h * GPC : (h + 1) * GPC],
                    k2T[mc][hr * HID : (hr + 1) * HID, :],
                    q2T[mc][hr * HID : (hr + 1) * HID, :],
                )
            e2 = wtile([P_D, H * GPC], "e2")
            for g in range(GPC):
                nc.vector.tensor_add(
                    e2[:, g : H * GPC : GPC],
                    s2ps[g * P_D : (g + 1) * P_D, g : H * GPC : GPC],
                    m2_sb[0:P_D, g : H * GPC : GPC],
                )
            nc.scalar.activation(e2[:], e2[:], AF.Exp, scale=0.125)

            # aggregate: per (graph, head-pair) one [32, 128] x [32, 2]
            # matmul. Column 4g+2mc+hh holds head (2mc+hh)'s output in rows
            # [64hh, 64hh+64); the other half is cross-head garbage, unread.
            o2 = ptile([128, H * GPC], "o2", tag="one")
            for g in range(GPC):
                for mc in range(2):
                    c0 = 4 * g + 2 * mc
                    nc.tensor.matmul(
                        o2[:, c0 : c0 + 2],
                        v2g[g][:, mc * 128 : (mc + 1) * 128],
                        e2[:, 8 * mc + g : 8 * mc + g + 5 : 4],
                    )
            z2ps = ptile([1, H * GPC], "z2ps", tag="one")
            nc.tensor.matmul(z2ps[:], ones_col[0:P_D, :], e2[:])
            z2row = wtile([1, H * GPC], "z2row")
            nc.vector.tensor_scalar_add(z2row[:], z2ps[:], EPS)
            rz2row = wtile([1, H * GPC], "rz2row")
            nc.vector.reciprocal(rz2row[:], z2row[:])
            rz2ps = ptile([HID, H * GPC], "rz2ps", tag="one")
            nc.tensor.matmul(rz2ps[:], ones_row[:, 0:HID], rz2row[:])
            rl_all = wtile([128, H * GPC], "rl_all")
            nc.scalar.activation(rl_all[:], o2[:], AF.Relu)
            h2T_f = []
            for mc in range(2):
                t = wtile([128, GPC], f"h2T_{mc}")
                for hh in range(2):
                    h = mc * 2 + hh
                    nc.vector.tensor_mul(
                        t[hh * HID : (hh + 1) * HID, :],
                        rl_all[
                            hh * HID : (hh + 1) * HID,
                            2 * mc + hh : H * GPC : 4,
                        ],
                        rz2ps[0:HID, h * GPC : (h + 1) * GPC],
                    )
                h2T_f.append(t)

            # ---- readout: out = h2 @ out_w + out_b ----
            ops = ptile([GPC, OUT], "outps", tag="one")
            nc.tensor.matmul(ops[:], h2T_f[0][:], wout0[:], start=True, stop=False)
            nc.tensor.matmul(ops[:], h2T_f[1][:], wout1[:], start=False, stop=False)
            nc.tensor.matmul(
                ops[:], ones_row[:, 0:GPC], woutb[:], start=False, stop=True
            )
            osb = wtile([GPC, OUT], "osb")
            nc.vector.tensor_copy(osb[:], ops[:])
            nc.sync.dma_start(out, osb[:])

        if hwloop and reps > 1:
            with tc.For_i(0, reps, 1):
                _rep_body()
        else:
            for _r in range(reps):
                _rep_body()

    nc.compile()
    return nc


# ----------------------------------------------------------------------------
# Entry point
# ----------------------------------------------------------------------------

def kernel(**inputs) -> np.ndarray:
    from concourse import bass_utils

    obs = np.asarray(inputs["obs"], np.float32)
    pre = _preprocess(obs, inputs["edge_index"])
    in_maps = _per_core_inputs(pre, inputs)

    key = (pre["P_D"], pre["P_S"])
    if key not in _COMPILE_CACHE:
        _COMPILE_CACHE[key] = _build_program(pre["P_D"], pre["P_S"], pre["SC"])
    nc = _COMPILE_CACHE[key]

    res = bass_utils.run_bass_kernel_spmd(nc, in_maps, core_ids=list(range(N_CORES)))
    out = np.concatenate([res.results[c]["out"] for c in range(N_CORES)], axis=0)
    return out.astype(np.float32)


# revision 20
# speedup vs baseline: 212.9611x; 7.9639x over previous
"""DGN network (encoder MLP -> 2x TransformerConv -> per-agent readout) on TRN2.

Strategy
--------
Data-parallel over the 32 independent subgraphs: 4 graphs per NeuronCore.
The final output reads conv2 only at each graph's agent node, so per graph
only the agent's 1-hop set D1 = {agent} u N(agent) and the 2-hop set
S1 = D1 u N(D1) participate.  The host builds the index lists, gathers the
participating node features (pure indexing), and builds exact additive
adjacency masks (log-multiplicity); the device runs small dense masked
attention on the gathered sets.  All matmuls, softmaxes and aggregations
run on hardware in fp32.

This platform charges a large fixed cost per compute-engine instruction
(DMAs are comparatively free), so the kernel is shaped to minimize the
number of PE/DVE/ACT instructions:
  - conv1 scores use host-precomputed A_h = Wk_aug_h Wq_aug_h^T so all
    heads' scores come from 4 tiny MMs (t = A h2a_d1) + SC wide MMs
    (s = h2a^T t), instead of separate q/k projections + 16 per-graph MMs.
  - the 4 graphs per core are treated as one dense node set; cross-graph
    score entries are killed exactly by the additive mask (exp -> 0).
  - conv1 aggregation contracts over 128-src chunks: H x SC accumulating
    MMs produce the row-major [(g,u), (h,65)] output directly (the 65th
    column of each head block is the softmax denominator, via a ones
    column packed into the V weights).
  - single wide PSUM->SBUF evacuations (3D access patterns) instead of
    per-chunk copies; biases ride the mandatory evacuation via
    scalar.activation(bias=...).
"""

from contextlib import ExitStack

import numpy as np

BS, N, D = 32, 1000, 6
HID, H = 64, 4
OUT = 5
N_CORES = 8
GPC = BS // N_CORES  # graphs per core
NEG8 = -8.0e30       # additive mask, pre-multiplied by sqrt(HID)=8
EPS = 1.0e-30

_COMPILE_CACHE: dict = {}


# ----------------------------------------------------------------------------
# Host-side preprocessing: index sets, gathered features, masks.
# ----------------------------------------------------------------------------

def _preprocess(obs: np.ndarray, edge_index: np.ndarray):
    obs = np.asarray(obs, dtype=np.float32)
    ei = np.asarray(edge_index)
    src = ei[0].astype(np.int64)
    dst = ei[1].astype(np.int64)

    node_feats = np.ascontiguousarray(obs[:, : N * 8].reshape(BS * N, 8)[:, 2:8])
    agent = np.clip(obs[:, -1], 0, N - 1).astype(np.int32)
    agent_glob = (np.arange(BS, dtype=np.int64) * N) + agent

    # CSR of in-edges (grouped by dst), multiplicity preserved
    order = np.argsort(dst, kind="stable")
    sdst = dst[order]
    ssrc = src[order]
    bounds = np.searchsorted(sdst, np.arange(BS * N + 1))

    def in_srcs(v):
        return ssrc[bounds[v] : bounds[v + 1]]

    D1_list, S1_list = [], []
    for b in range(BS):
        a = int(agent_glob[b])
        nbr = in_srcs(a)
        others = np.unique(nbr)
        others = others[others != a]
        D1 = np.concatenate([[a], others]).astype(np.int64)
        srcs_all = np.unique(np.concatenate([in_srcs(int(u)) for u in D1]))
        extra = np.setdiff1d(srcs_all, D1)
        S1 = np.concatenate([D1, extra])
        D1_list.append(D1)
        S1_list.append(S1)

    max_d1 = max(len(x) for x in D1_list)
    max_s1 = max(len(x) for x in S1_list)
    P_D = 32 * ((max_d1 + 31) // 32)
    P_S = 32 * ((max_s1 + 31) // 32)
    assert P_D <= 32, f"agent degree too large for this layout: |D1|={max_d1}"
    assert P_S <= 512, f"2-hop set too large: |S1|={max_s1}"
    SC = GPC * P_S // 128      # 128-src chunks per core (WT = GPC*P_S)

    # Per-graph gather indices (padded with 0 -> harmless real data, masked)
    gidx = np.zeros((BS, P_S), np.int32)
    # conv1 mask, [S1 position, D1 position], pre-scaled by 8
    m1t8 = np.full((BS, P_S, P_D), NEG8, np.float32)
    # conv2 mask over D1 source positions
    m2t8 = np.full((BS, P_D), NEG8, np.float32)

    for b in range(BS):
        D1, S1 = D1_list[b], S1_list[b]
        gidx[b, : len(S1)] = S1
        pos = {int(v): i for i, v in enumerate(S1)}
        for up, u in enumerate(D1):
            s_of_u = in_srcs(int(u))
            if len(s_of_u) == 0:
                continue
            vals, cnts = np.unique(s_of_u, return_counts=True)
            for v, c in zip(vals, cnts):
                m1t8[b, pos[int(v)], up] = 8.0 * np.log(np.float32(c)) if c > 1 else 0.0
        a_srcs = in_srcs(int(D1[0]))
        if len(a_srcs):
            vals, cnts = np.unique(a_srcs, return_counts=True)
            for v, c in zip(vals, cnts):
                sp = pos[int(v)]
                assert sp < len(D1)
                m2t8[b, sp] = 8.0 * np.log(np.float32(c)) if c > 1 else 0.0

    return dict(
        node_feats=node_feats,
        gidx=gidx,
        m1t8=m1t8,
        m2t8=m2t8,
        P_D=P_D,
        P_S=P_S,
        SC=SC,
    )


def _pack_v_weights(wv, bv, n_in):
    """[n_in+1, 65*H] tile: head h -> cols [65h:65h+64] = Wv head block (with
    bias row at n_in); col 65h+64 = basis vector selecting the ones row, so
    the AV matmul also produces the softmax denominator."""
    p = np.zeros((n_in + 1, 65 * H), np.float32)
    for h in range(H):
        p[:n_in, 65 * h : 65 * h + HID] = wv[:, HID * h : HID * (h + 1)]
        p[n_in, 65 * h : 65 * h + HID] = bv[HID * h : HID * (h + 1)]
        p[n_in, 65 * h + HID] = 1.0
    return p


SMALL_SPECS = [("w1", D, HID), ("w2", HID, HID), ("b12", HID, 2)]


def _small_layout():
    layout, c = {}, 0
    for name, rows, cols in SMALL_SPECS:
        layout[name] = (c, c + cols, rows)
        c += cols
    layout["_total"] = c
    return layout


def _pack_layout(SC):
    """Column layout of the single consolidated [128, X] weight/mask pack."""
    specs = [
        ("ident", 128, 128),
        ("ones", 1, 512),
        ("onescol", 128, 1),
        ("a1t", HID + 1, H * (HID + 1)),
        ("wv1p", HID + 1, 65 * H),
        ("q2_k0", 128, H * HID), ("q2_k1", 128, H * HID), ("q2_bp", 128, 2),
        ("k2_k0", 128, H * HID), ("k2_k1", 128, H * HID), ("k2_bp", 128, 2),
        ("v2_k0", 128, H * HID), ("v2_k1", 128, H * HID), ("v2_kb", 1, H * HID),
        ("wout0", 128, OUT), ("wout1", 128, OUT), ("woutb", 1, OUT),
        ("m1big", 128, SC * 512),
        ("m2full", 128, H * GPC),
    ]
    layout, c = {}, 0
    for name, rows, cols in specs:
        layout[name] = (c, c + cols, rows)
        c += cols
    layout["_total"] = c
    return layout


def _per_core_inputs(pre, weights):
    P_D, P_S, SC = pre["P_D"], pre["P_S"], pre["SC"]
    WT = GPC * P_S
    w = weights
    layout = _pack_layout(SC)
    base = np.zeros((128, layout["_total"]), np.float32)
    slayout = _small_layout()
    small = np.zeros((128, slayout["_total"]), np.float32)

    def put(name, arr):
        c0, c1, rows = layout[name]
        assert arr.shape == (rows, c1 - c0), (name, arr.shape)
        base[:rows, c0:c1] = arr

    def puts(name, arr):
        c0, c1, rows = slayout[name]
        assert arr.shape == (rows, c1 - c0), (name, arr.shape)
        small[:rows, c0:c1] = arr

    put("ident", np.eye(128, dtype=np.float32))
    put("ones", np.ones((1, 512), np.float32))
    put("onescol", np.ones((128, 1), np.float32))

    # conv1 score kernels: lhsT for t = A_h h2a is A_h^T = Wq_aug Wk_aug^T
    wq1 = np.asarray(w["c1_wq"], np.float32)
    bq1 = np.asarray(w["c1_bq"], np.float32)
    wk1 = np.asarray(w["c1_wk"], np.float32)
    bk1 = np.asarray(w["c1_bk"], np.float32)
    a1t = np.zeros((HID + 1, H * (HID + 1)), np.float32)
    for h in range(H):
        wqa = np.vstack([wq1[:, HID * h : HID * (h + 1)],
                         bq1[None, HID * h : HID * (h + 1)]])  # [65, 64]
        wka = np.vstack([wk1[:, HID * h : HID * (h + 1)],
                         bk1[None, HID * h : HID * (h + 1)]])
        a1t[:, (HID + 1) * h : (HID + 1) * (h + 1)] = wqa @ wka.T
    put("a1t", a1t)
    put("wv1p", _pack_v_weights(
        np.asarray(w["c1_wv"], np.float32), np.asarray(w["c1_bv"], np.float32), HID
    ))

    wq2a = np.vstack([w["c2_wq"], w["c2_bq"][None, :]]).astype(np.float32)
    wk2a = np.vstack([w["c2_wk"], w["c2_bk"][None, :]]).astype(np.float32)
    wv2a = np.vstack([w["c2_wv"], w["c2_bv"][None, :]]).astype(np.float32)
    wouta = np.vstack([w["out_w"], w["out_b"][None, :]]).astype(np.float32)

    puts("w1", np.asarray(w["enc_w1"], np.float32))
    puts("w2", np.asarray(w["enc_w2"], np.float32))
    puts("b12", np.stack([w["enc_b1"], w["enc_b2"]], axis=1).astype(np.float32))
    for nm, arr in (("q2", wq2a), ("k2", wk2a)):
        put(f"{nm}_k0", arr[0:128])
        put(f"{nm}_k1", arr[128:256])
        put(f"{nm}_bp", arr[256].reshape(2, 128).T.copy())
    put("v2_k0", wv2a[0:128])
    put("v2_k1", wv2a[128:256])
    put("v2_kb", wv2a[256:257])
    put("wout0", wouta[0:128])
    put("wout1", wouta[128:256])
    put("woutb", wouta[256:257])

    in_maps = []
    for c in range(N_CORES):
        pack = base.copy()

        def putc(name, arr):
            c0, c1, rows = layout[name]
            assert arr.shape == (rows, c1 - c0), (name, arr.shape)
            pack[:rows, c0:c1] = arr

        gs = slice(c * GPC, (c + 1) * GPC)
        # host-side gather: featsT [7, WT], row 6 = ones (bias row)
        flat = pre["gidx"][gs].reshape(-1).astype(np.int64)   # [WT]
        gf = pre["node_feats"][flat]                          # [WT, 6]
        featsT = np.ones((D + 1, WT), np.float32)
        featsT[0:D] = gf.T
        # conv1 mask, chunk-dense: [128 src-in-chunk, (chunk, head, g, u)]
        m1big = np.full((128, SC * 512), NEG8, np.float32)
        for ck in range(SC):
            for p in range(128):
                s = ck * 128 + p
                if s >= WT:
                    break
                g_s, pos = divmod(s, P_S)
                row = pre["m1t8"][c * GPC + g_s][pos]         # [P_D]
                for h in range(H):
                    col0 = ck * 512 + h * 128 + g_s * P_D
                    m1big[p, col0 : col0 + P_D] = row
        putc("m1big", m1big)
        # conv2 mask [P_D (D1 source position), (h, g)]
        m2full = np.full((128, H * GPC), NEG8, np.float32)
        for g in range(GPC):
            for h in range(H):
                m2full[:P_D, h * GPC + g] = pre["m2t8"][c * GPC + g]
        putc("m2full", m2full)
        in_maps.append({"featsT": featsT, "wpack": pack, "wsmall": small})
    return in_maps


# ----------------------------------------------------------------------------
# Device program
# ----------------------------------------------------------------------------

def _build_program(P_D, P_S, SC, reps=1, enable_asserts=False, hwloop=True):
    import concourse.bass as bass
    import concourse.tile as tile
    from concourse import bacc, mybir

    f32 = mybir.dt.float32
    AF = mybir.ActivationFunctionType

    assert P_D == 32 and GPC == 4
    UW = GPC * P_D              # packed conv1-dst width = 128
    WT = GPC * P_S              # gathered-node columns
    assert WT % 128 == 0 and SC == WT // 128
    assert WT <= 512, f"encoder single-chunk layout needs WT<=512, got {WT}"

    nc = bacc.Bacc(
        "TRN2",
        target_bir_lowering=False,
        debug=False,
        enable_asserts=enable_asserts,
        num_devices=N_CORES,
    )

    layout = _pack_layout(SC)
    tot_cols = layout["_total"]
    featsT = nc.dram_tensor("featsT", (D + 1, WT), f32, kind="ExternalInput").ap()
    wpack = nc.dram_tensor("wpack", (128, tot_cols), f32, kind="ExternalInput").ap()
    slayout = _small_layout()
    wsmall = nc.dram_tensor(
        "wsmall", (128, slayout["_total"]), f32, kind="ExternalInput"
    ).ap()
    out = nc.dram_tensor("out", (GPC, OUT), f32, kind="ExternalOutput").ap()

    with tile.TileContext(nc) as tc, ExitStack() as ctx:
        cp = ctx.enter_context(tc.tile_pool(name="const", bufs=1))
        wp = ctx.enter_context(tc.tile_pool(name="work", bufs=2))
        pp = ctx.enter_context(tc.tile_pool(name="psum", bufs=1, space="PSUM"))

        def ctile(shape, name, dt=f32):
            return cp.tile(shape, dt, tag=name, name=name)

        wsm = ctile([128, slayout["_total"]], "wsm")
        nc.sync.dma_start(wsm[:], wsmall)
        wpk = ctile([128, tot_cols], "wpk")
        nc.sync.dma_start(wpk[:], wpack)

        def wsl(name, rows):
            c0, c1, _r = layout[name]
            return wpk[0:rows, c0:c1]

        def ssl(name, rows):
            c0, c1, _r = slayout[name]
            return wsm[0:rows, c0:c1]

        ident = wsl("ident", 128)
        ones_row = wsl("ones", 1)
        ones_col = wsl("onescol", 128)
        w1_sb = ssl("w1", D)
        w2_sb = ssl("w2", HID)
        b12_sb = ssl("b12", HID)
        a1t_sb = wsl("a1t", HID + 1)
        wv1_sb = wsl("wv1p", HID + 1)
        w2ch = {
            nm: (wsl(f"{nm}_k0", 128), wsl(f"{nm}_k1", 128), wsl(f"{nm}_bp", 128))
            for nm in ("q2", "k2")
        }
        w2ch["v2"] = (wsl("v2_k0", 128), wsl("v2_k1", 128), wsl("v2_kb", 1))
        wout0 = wsl("wout0", 128)
        wout1 = wsl("wout1", 128)
        woutb = wsl("woutb", 1)
        m1_sb = wsl("m1big", 128)
        m2_sb = wsl("m2full", 128)
        ones_c0, _oc1, _ocr = layout["ones"]

        def _rep_body():
            def wtile(shape, name, dt=f32):
                return wp.tile(shape, dt, tag=name, name=name)

            def ptile(shape, name, tag, bufs=2):
                return pp.tile(shape, f32, tag=tag, name=name, bufs=bufs)

            # ---- load gathered features (host did the gather) ----
            fT = wtile([D + 1, WT], "fT")
            nc.sync.dma_start(fT[:], featsT)

            # ---- encoder: 2 MMs + 2 ACTs over all WT columns at once ----
            h2a = wtile([HID + 1, WT], "h2a")
            # ones row for the bias/denominator tricks comes via DMA
            nc.sync.dma_start(
                h2a[HID : HID + 1, :], wpack[0:1, ones_c0 : ones_c0 + WT]
            )
            p1 = ptile([HID, 512], "h1ps", tag="one")
            nc.tensor.matmul(p1[:, 0:WT], w1_sb[:], fT[0:D, :])
            h1T = wtile([HID, WT], "h1T")
            nc.scalar.activation(h1T[:], p1[:, 0:WT], AF.Relu, bias=b12_sb[:, 0:1])
            p2 = ptile([HID, 512], "h2ps", tag="one")
            nc.tensor.matmul(p2[:, 0:WT], w2_sb[:], h1T[:])
            nc.scalar.activation(
                h2a[0:HID, :], p2[:, 0:WT], AF.Relu, bias=b12_sb[:, 1:2]
            )
            h2a_d1 = h2a.rearrange("p (g c) -> p g c", g=GPC)[:, :, 0:P_D]

            # ---- conv1 scores: t = A_h h2a_d1 (4 MMs), s = h2a^T t ----
            tps = ptile([HID + 1, 512], "tps", tag="one")
            for h in range(H):
                nc.tensor.matmul(
                    tps[:, UW * h : UW * (h + 1)],
                    a1t_sb[:, (HID + 1) * h : (HID + 1) * (h + 1)],
                    h2a_d1,
                )
            t_sb = wtile([HID + 1, 512], "t_sb")
            nc.scalar.copy(t_sb[:], tps[:])

            sps = ptile([128, SC * 512], "sps", tag="big")
            for c in range(SC):
                nc.tensor.matmul(
                    sps[:, 512 * c : 512 * (c + 1)],
                    h2a[:, 128 * c : 128 * (c + 1)],
                    t_sb[:],
                )
            e_sb = wtile([128, SC * 512], "e_sb")
            nc.vector.tensor_add(e_sb[:], sps[:], m1_sb[:, 0 : SC * 512])
            nc.scalar.activation(e_sb[:], e_sb[:], AF.Exp, scale=0.125)

            # ---- conv1 v (per src chunk) ----
            vps = ptile([128, SC * 512], "vps", tag="big")
            for c in range(SC):
                nc.tensor.matmul(
                    vps[:, 512 * c : 512 * c + 65 * H],
                    h2a[:, 128 * c : 128 * (c + 1)],
                    wv1_sb[:],
                )
            v_sb = wtile([128, SC * 65 * H], "v_sb")
            nc.scalar.copy(
                v_sb.rearrange("p (c w) -> p c w", c=SC),
                vps.rearrange("p (c w) -> p c w", c=SC)[:, :, 0 : 65 * H],
            )

            # ---- conv1 aggregation: H x SC accumulating MMs ----
            o1 = ptile([128, 512], "o1", tag="one")
            for h in range(H):
                for c in range(SC):
                    nc.tensor.matmul(
                        o1[:, 65 * h : 65 * h + 65],
                        e_sb[:, 512 * c + UW * h : 512 * c + UW * (h + 1)],
                        v_sb[:, 65 * H * c + 65 * h : 65 * H * c + 65 * h + 65],
                        start=(c == 0),
                        stop=(c == SC - 1),
                    )

            # normalization: per-partition (per dst node) activation scale
            z1 = wtile([128, H], "z1")
            nc.vector.tensor_scalar_add(z1[:], o1[:, HID : 65 * H : 65], EPS)
            rz1 = wtile([128, H], "rz1")
            nc.vector.reciprocal(rz1[:], z1[:])
            h1cRM = wtile([128, H * HID], "h1cRM")
            for h in range(H):
                nc.scalar.activation(
                    h1cRM[:, HID * h : HID * (h + 1)],
                    o1[:, 65 * h : 65 * h + HID],
                    AF.Relu,
                    scale=rz1[:, h : h + 1],
                )

            # transpose h1c to feature-major for the conv2 projections
            h1cT = []
            for mc in range(2):
                tp = ptile([128, 128], "h1cTps", tag="one")
                nc.tensor.transpose(
                    tp[:], h1cRM[:, 128 * mc : 128 * (mc + 1)], ident[:]
                )
                t = wtile([128, 128], f"h1cT_{mc}")
                nc.scalar.copy(t[:], tp[:])
                h1cT.append(t)

            # ---- conv2 projections (agents / D1 nodes only) ----
            agent_cols_a = h1cT[0][:, 0:UW:P_D]
            agent_cols_b = h1cT[1][:, 0:UW:P_D]

            def proj2(nm, rhs_a, rhs_b, width, name):
                k0, k1_, bp = w2ch[nm]
                outt = []
                for mc in range(2):
                    ps = ptile([128, width], f"{name}ps_{mc}", tag="one")
                    nc.tensor.matmul(
                        ps[:], k0[:, mc * 128 : (mc + 1) * 128],
                        rhs_a, start=True, stop=False,
                    )
                    nc.tensor.matmul(
                        ps[:], k1_[:, mc * 128 : (mc + 1) * 128],
                        rhs_b, start=False, stop=True,
                    )
                    t = wtile([128, width], f"{name}_{mc}")
                    nc.scalar.activation(
                        t[:], ps[:], AF.Identity, bias=bp[:, mc : mc + 1]
                    )
                    outt.append(t)
                return outt

            q2T = proj2("q2", agent_cols_a, agent_cols_b, GPC, "q2T")
            k2T = proj2("k2", h1cT[0][:], h1cT[1][:], UW, "k2T")

            # v2 for all D1 nodes at once: [128, 256]
            vps2 = ptile([128, H * HID], "v2ps", tag="one")
            vk0, vk1, vkb = w2ch["v2"]
            nc.tensor.matmul(vps2[:], h1cT[0][:], vk0[:], start=True, stop=False)
            nc.tensor.matmul(vps2[:], h1cT[1][:], vk1[:], start=False, stop=False)
            nc.tensor.matmul(
                vps2[:], ones_row[:, 0:UW], vkb[:], start=False, stop=True
            )
            v2g = []
            for g in range(GPC):
                t = wtile([P_D, H * HID], f"v2_{g}")
                nc.vector.tensor_copy(t[:], vps2[g * P_D : (g + 1) * P_D, :])
                v2g.append(t)

            # ---- conv2 attention: merged scores [128 (all D1), H*GPC] ----
            s2ps = ptile([128, H * GPC], "s2ps", tag="one")
            for h in range(H):
                mc, hr = divmod(h, 2)
                nc.tensor.matmul(
                    s2ps[:, h * GPC : (h + 1) * GPC],
                    k2T[mc][hr * HID : (hr + 1) * HID, :],
                    q2T[mc][hr * HID : (hr + 1) * HID, :],
                )
            e2 = wtile([P_D, H * GPC], "e2")
            for g in range(GPC):
                nc.vector.tensor_add(
                    e2[:, g : H * GPC : GPC],
                    s2ps[g * P_D : (g + 1) * P_D, g : H * GPC : GPC],
                    m2_sb[0:P_D, g : H * GPC : GPC],
                )
            nc.scalar.activation(e2[:], e2[:], AF.Exp, scale=0.125)

            # aggregate: per (graph, head-pair) one [32, 128] x [32, 2]
            # matmul. Column 4g+2mc+hh holds head (2mc+hh)'s output in rows
            # [64hh, 64hh+64); the other half is cross-head garbage, unread.
            o2 = ptile([128, H * GPC], "o2", tag="one")
            for g in range(GPC):
                for mc in range(2):
                    c0 = 4 * g + 2 * mc
                    nc.tensor.matmul(
                        o2[:, c0 : c0 + 2],
                        v2g[g][:, mc * 128 : (mc + 1) * 128],
                        e2[:, 8 * mc + g : 8 * mc + g + 5 : 4],
                    )
            z2ps = ptile([1, H * GPC], "z2ps", tag="one")
            nc.tensor.matmul(z2ps[:], ones_col[0:P_D, :], e2[:])
            z2row = wtile([1, H * GPC], "z2row")
            nc.vector.tensor_scalar_add(z2row[:], z2ps[:], EPS)
            rz2row = wtile([1, H * GPC], "rz2row")
            nc.vector.reciprocal(rz2row[:], z2row[:])
            rz2ps = ptile([HID, H * GPC], "rz2ps", tag="one")
            nc.tensor.matmul(rz2ps[:], ones_row[:, 0:HID], rz2row[:])
            rl_all = wtile([128, H * GPC], "rl_all")
            nc.scalar.activation(rl_all[:], o2[:], AF.Relu)
            h2T_f = []
            for mc in range(2):
                t = wtile([128, GPC], f"h2T_{mc}")
                for hh in range(2):
                    h = mc * 2 + hh
                    nc.vector.tensor_mul(
                        t[hh * HID : (hh + 1) * HID, :],
                        rl_all[
                            hh * HID : (hh + 1) * HID,
                            2 * mc + hh : H * GPC : 4,
                        ],
                        rz2ps[0:HID, h * GPC : (h + 1) * GPC],
                    )
                h2T_f.append(t)

            # ---- readout: out = h2 @ out_w + out_b ----
            ops = ptile([GPC, OUT], "outps", tag="one")
            nc.tensor.matmul(ops[:], h2T_f[0][:], wout0[:], start=True, stop=False)
            nc.tensor.matmul(ops[:], h2T_f[1][:], wout1[:], start=False, stop=False)
            nc.tensor.matmul(
                ops[:], ones_row[:, 0:GPC], woutb[:], start=False, stop=True
            )
            osb = wtile([GPC, OUT], "osb")
            nc.vector.tensor_copy(osb[:], ops[:])
            nc.sync.dma_start(out, osb[:])

        if hwloop and reps > 1:
            with tc.For_i(0, reps, 1):
                _rep_body()
        else:
            for _r in range(reps):
                _rep_body()

    nc.compile()
    return nc


# ----------------------------------------------------------------------------
# Entry point
# ----------------------------------------------------------------------------

def kernel(**inputs) -> np.ndarray:
    from concourse import bass_utils

    obs = np.asarray(inputs["obs"], np.float32)
    pre = _preprocess(obs, inputs["edge_index"])
    in_maps = _per_core_inputs(pre, inputs)

    key = (pre["P_D"], pre["P_S"])
    if key not in _COMPILE_CACHE:
        _COMPILE_CACHE[key] = _build_program(pre["P_D"], pre["P_S"], pre["SC"])
    nc = _COMPILE_CACHE[key]

    res = bass_utils.run_bass_kernel_spmd(nc, in_maps, core_ids=list(range(N_CORES)))
    out = np.concatenate([res.results[c]["out"] for c in range(N_CORES)], axis=0)
    return out.astype(np.float32)
